# revision 1
# baseline (speedup 1.0000x reference)
"""Trainium2 Bass kernel for nn_MultiHeadAttention_38027640439053.

Reference computation (per batch b of 8, one NeuronCore each):
    data = X.reshape(n, 16, 64)
    q/k/v = data @ W{q,k,v}.T          (per-head shared 64x64 weights)
    scores = (q @ k.T per head) / 32
    attn = softmax(scores, axis=k)
    Y = (attn @ v).reshape(n, 1024) @ Wo.T + bo

Kernel strategy (batch-parallel over 8 cores, zero collectives):
  - X is transposed on-chip via PE transposes (XT: emb on partitions).
  - Q/K projected two-heads-at-a-time with block-diag(W.T) stationary
    weights -> QT/KT in [head_dim, n] layout.
  - Scores computed TRANSPOSED: ST[k, q] = K Q^T per head, two heads
    run concurrently in the PE array via row-tiling (K=64 each).
  - exp() on ScalarE directly from PSUM with the 1/32 scale folded in.
    No max-subtraction (scores have sigma ~0.25; exp range ~[0.2, 5]).
  - P@V computed as YT = V^T P^T with V in row layout augmented by a
    ones column -> row 64 of the output is the softmax denominator D.
  - Normalization deferred: recip(D) on VectorE, broadcast across
    partitions with a 0/1 selector matmul (full fp32), one multiply
    per slab.
  - Software pipelining: the ST+exp phase of pair p is ACT-bound
    (ScalarE exp is ~2.3x slower than the PE making scores), so pair
    p-1's P@V matmuls and pair p+1's projections are interleaved into
    pair p's score loop to keep the PE busy.
  - Output projection accumulates over 8 emb chunks per n-tile; the
    bias is added with a rank-1 (ones x bo) matmul into the same PSUM
    accumulation group. Wo^T streams into recycled PT slots at the
    tail.

float32r (reduced-precision fp32 PE mode, ~TF32 accuracy, 4x faster
than true fp32 for free dim >= 256) is used for the large matmuls.
"""

import numpy as np

import concourse.bacc as bacc
import concourse.mybir as mybir
import concourse.tile as tile
from concourse.bass_utils import run_bass_kernel_spmd

F32 = mybir.dt.float32
F32R = mybir.dt.float32r

EXP = mybir.ActivationFunctionType.Exp

# stages using float32r ("fast" reduced-precision fp32 matmul mode)
DEFAULT_FAST = frozenset({"proj", "st", "pv", "outp", "bias", "bcast"})


def emit_body(tc, nc, aps, N, EMB, NH, fast, rep):
    NPAIR = NH // 2
    NT = N // 128        # n tiles (rows of X / q tiles)
    KT = N // 128        # k tiles
    assert EMB == NPAIR * 128
    scale = 1.0 / float(np.sqrt(EMB))
    qch = [(s, min(512, N - s)) for s in range(0, N, 512)]
    ech = [(s, min(512, EMB - s)) for s in range(0, EMB, 512)]
    KT_PER = min(4, KT)
    assert KT % KT_PER == 0
    NPT = KT // KT_PER   # PT tiles per head

    X_d, WqT2_d, WkT2_d, WvT2_d, WoT_d, bo_d, sel_d, ones_d, iden_d, Y_d = aps

    dt_w = F32R if "proj" in fast else F32    # wq2/wk2/wv2 + xt
    dt_qk = F32R if "st" in fast else F32     # qt/kt
    dt_pv = F32R if "pv" in fast else F32     # pt/vslab
    dt_o = F32R if "outp" in fast else F32    # yt/wot
    dt_b = F32R if "bias" in fast else F32    # ones/bo
    dt_r = F32R if "bcast" in fast else F32   # sel/rd/dh

    with (
        tc.tile_pool(name=f"consts{rep}", bufs=1) as consts,
        tc.tile_pool(name=f"xp{rep}", bufs=3) as xp,
        tc.tile_pool(name=f"xtp{rep}", bufs=2) as xtp,
        tc.tile_pool(name=f"qkp{rep}", bufs=3) as qkp,
        tc.tile_pool(name=f"vp{rep}", bufs=2) as vp,
        tc.tile_pool(name=f"ptp{rep}", bufs=5) as ptp,
        tc.tile_pool(name=f"ytp{rep}", bufs=NPAIR) as ytp,
        tc.tile_pool(name=f"rdp{rep}", bufs=1) as rdp,
        tc.tile_pool(name=f"stps{rep}", bufs=2, space="PSUM") as stps,
        tc.tile_pool(name=f"mps{rep}", bufs=2, space="PSUM") as mps,
    ):
        # ---- constants ----
        iden = consts.tile([128, 128], F32, name="iden", tag="iden")
        nc.sync.dma_start(out=iden[:], in_=iden_d[:])
        wq2 = consts.tile([128, 128], dt_w, name="wq2", tag="wq2")
        nc.sync.dma_start(out=wq2[:], in_=WqT2_d[:])
        wk2 = consts.tile([128, 128], dt_w, name="wk2", tag="wk2")
        nc.sync.dma_start(out=wk2[:], in_=WkT2_d[:])
        wv2 = consts.tile([128, 128], dt_w, name="wv2", tag="wv2")
        nc.sync.dma_start(out=wv2[:], in_=WvT2_d[:])
        bo_t = consts.tile([1, EMB], dt_b, name="bo_t", tag="bo_t")
        nc.sync.dma_start(out=bo_t[:], in_=bo_d[:])
        ones_t = consts.tile([1, 128], dt_b, name="ones_t", tag="ones_t")
        nc.sync.dma_start(out=ones_t[:], in_=ones_d[:])

        # ---- X column slabs, loaded just in time per pair ----
        # slab p holds X[:, p*128:(p+1)*128] as [128 rows-of-ntile, NT*128]:
        # [part, i*128 + c] = X[i*128 + part, p*128 + c]
        x_slabs = {}

        def load_x(p):
            xs = xp.tile([128, NT * 128], F32, name=f"x{p}", tag="x")
            nc.sync.dma_start(
                out=xs[:].rearrange("p (i c) -> p i c", c=128),
                in_=X_d[:, p * 128:(p + 1) * 128]
                .rearrange("(i p) c -> p i c", p=128))
            x_slabs[p] = xs

        # selector needs its own slot: it is read by bcast_mul(0) early in
        # the pipeline, long before the X tiles die.
        sel_t = xp.tile([NH, EMB], dt_r, name="sel_t", tag="selx")
        nc.sync.dma_start(out=sel_t[:], in_=sel_d[:])

        rd = rdp.tile([NH, N], dt_r, name="rd", tag="rd")
        # zero so early per-pair selector matmuls read 0 (not garbage/NaN)
        # from rows of rd that a later pair will write. (ACT copy with
        # scale=0 instead of memset: memset cannot write float32r.)
        nc.scalar.activation(rd[:], sel_t[:, 0:N],
                             mybir.ActivationFunctionType.Copy,
                             bias=0.0, scale=0.0)

        def build_xt(p):
            """XT slab p: [128 emb dims of pair p, N] via 8 PE transposes."""
            xt_ps = mps.tile([128, N], F32, name=f"xtps{p}", tag="m")
            xs = x_slabs.pop(p)
            for i in range(NT):
                nc.tensor.transpose(
                    xt_ps[:, i * 128:(i + 1) * 128],
                    xs[:, i * 128:(i + 1) * 128],
                    iden[:],
                )
            xt = xtp.tile([128, N], dt_w, name=f"xt{p}", tag="xt")
            nc.vector.tensor_copy(xt[:], xt_ps[:])
            return xt

        def proj_qk(p, xt):
            qps = mps.tile([128, N], F32, name=f"qps{p}", tag="m")
            for (s, w) in qch:
                nc.tensor.matmul(qps[:, s:s + w], wq2[:], xt[:, s:s + w])
            qt = qkp.tile([128, N], dt_qk, name=f"qt{p}", tag="qk")
            nc.vector.tensor_copy(qt[:], qps[:])

            kps = mps.tile([128, N], F32, name=f"kps{p}", tag="m")
            for (s, w) in qch:
                nc.tensor.matmul(kps[:, s:s + w], wk2[:], xt[:, s:s + w])
            kt = qkp.tile([128, N], dt_qk, name=f"kt{p}", tag="qk")
            nc.vector.tensor_copy(kt[:], kps[:])
            return qt, kt

        def proj_v(p, xt):
            # V in row layout: [n, 2 heads x 64]
            vps = mps.tile([128, N], F32, name=f"vps{p}", tag="m")
            for i in range(NT):
                nc.tensor.matmul(vps[:, i * 128:(i + 1) * 128],
                                 xt[:, i * 128:(i + 1) * 128], wv2[:])
            vslab = vp.tile([128, KT * 130], dt_pv, name=f"vslab{p}", tag="v")
            vv = vslab[:].rearrange("p (j c) -> p j c", c=130)
            vs = vps[:].rearrange("p (j c) -> p j c", c=128)
            nc.vector.tensor_copy(vv[:, :, 0:64], vs[:, :, 0:64])
            nc.vector.tensor_copy(vv[:, :, 65:129], vs[:, :, 64:128])
            v4 = vslab[:].rearrange("p (j k c) -> p j k c", k=2, c=65)
            ones_src = iden[:, 0:2 * KT].rearrange("p (j k c) -> p j k c",
                                                   k=2, c=1)
            nc.scalar.activation(v4[:, :, :, 64:65], ones_src,
                                 mybir.ActivationFunctionType.Copy,
                                 bias=1.0, scale=0.0)
            return vslab

        def st_exp(p, ktile, qt, kt, pts):
            """Scores (transposed) + exp for one k-tile, both heads."""
            for head in (0, 1):
                r0 = head * 64
                st = stps.tile([128, N], F32, name=f"st{p}_{ktile}_{head}",
                               tag="st")
                for (s, w) in qch:
                    nc.tensor.matmul(
                        st[:, s:s + w],
                        kt[r0:r0 + 64, ktile * 128:(ktile + 1) * 128],
                        qt[r0:r0 + 64, s:s + w],
                    )
                if ktile % KT_PER == 0:
                    pt = ptp.tile([128, KT_PER * N], dt_pv,
                                  name=f"pt{p}_{head}_{ktile // KT_PER}",
                                  tag="pt")
                    pts[head].append(pt)
                dst = pts[head][-1][:, (ktile % KT_PER) * N:
                                    (ktile % KT_PER + 1) * N]
                nc.scalar.activation(dst, st[:], EXP, scale=scale)

        # pipeline state for the deferred PV of the previous pair
        pv_state = {}

        def pv_quarter(p, head, half, vslab, pts):
            """8 accumulating matmuls: chunks [half*KT/2, (half+1)*KT/2)."""
            if half == 0:
                pv_state[(p, head)] = mps.tile(
                    [65, N], F32, name=f"pvps{p}_{head}", tag="m")
            pvps = pv_state[(p, head)]
            k0, k1 = half * (KT // 2), (half + 1) * (KT // 2)
            for ktile in range(k0, k1):
                pt = pts[head][ktile // KT_PER]
                base = (ktile % KT_PER) * N
                lhs = vslab[:, ktile * 130 + head * 65:
                            ktile * 130 + head * 65 + 65]
                for (s, w) in qch:
                    nc.tensor.matmul(
                        pvps[:, s:s + w], lhs,
                        pt[:, base + s:base + s + w],
                        start=(ktile == 0), stop=(ktile == KT - 1),
                    )

        def finish_head(p, head, yt):
            pvps = pv_state.pop((p, head))
            nc.vector.tensor_copy(yt[head * 64:head * 64 + 64, :],
                                  pvps[0:64, :])
            dh = xtp.tile([65, N], dt_r, name=f"dh{p}_{head}", tag="xt")
            with nc.allow_low_precision(reason="f32r softmax denom"):
                nc.vector.reciprocal(dh[64:65, :], pvps[64:65, :])
            nc.sync.dma_start(out=rd[2 * p + head:2 * p + head + 1, :],
                              in_=dh[64:65, :])

        def bcast_mul(p, yt):
            bps = mps.tile([128, N], F32, name=f"bps{p}", tag="m")
            for (s, w) in qch:
                nc.tensor.matmul(bps[:, s:s + w],
                                 sel_t[:, p * 128:(p + 1) * 128],
                                 rd[:, s:s + w])
            nc.vector.tensor_mul(yt[:], yt[:], bps[:])

        # ---------------- pipelined pair loop ----------------
        yts = []
        all_pts = {}
        vslabs = {}

        # pair 0 prologue: xt0 built once, q/k/v projections
        load_x(0)
        xt0 = build_xt(0)
        cur_qt, cur_kt = proj_qk(0, xt0)
        vslabs[0] = proj_v(0, xt0)
        nxt = {}
        for p in range(NPAIR):
            pts = {0: [], 1: []}
            all_pts[p] = pts
            yts.append(ytp.tile([128, N], dt_o, name=f"yt{p}", tag="yt"))

            sched = {k: [] for k in range(KT)}
            if p > 0:
                po, vo, pp = p - 1, vslabs[p - 1], all_pts[p - 1]
                yo = yts[p - 1]
                tasks = [
                    lambda: pv_quarter(po, 0, 0, vo, pp),
                    lambda: (pv_quarter(po, 0, 1, vo, pp),
                             finish_head(po, 0, yo)),
                    lambda: pv_quarter(po, 1, 0, vo, pp),
                    lambda: (pv_quarter(po, 1, 1, vo, pp),
                             finish_head(po, 1, yo), bcast_mul(po, yo)),
                ]
                for j, pos in enumerate((0, KT // 4, KT // 2,
                                         (3 * KT) // 4)):
                    sched[min(KT - 1, pos)].append(tasks[j])
            if p + 1 < NPAIR:
                pn = p + 1
                tasks = [
                    lambda: load_x(pn),
                    lambda: nxt.__setitem__("xt", build_xt(pn)),
                    lambda: nxt.__setitem__("qk", proj_qk(pn, nxt["xt"])),
                    lambda: vslabs.__setitem__(pn, proj_v(pn, nxt["xt"])),
                ]
                for j, pos in enumerate((0, KT - 5, KT - 3, KT - 1)):
                    sched[max(0, pos)].append(tasks[j])
            for ktile in range(KT):
                for t in sched[ktile]:
                    t()
                st_exp(p, ktile, cur_qt, cur_kt, pts)
            if p - 1 >= 0:
                del vslabs[p - 1], all_pts[p - 1]
            if p + 1 < NPAIR:
                cur_qt, cur_kt = nxt["qk"]

        # ---------------- tail: last pair's PV + Wo load + outproj ----
        last = NPAIR - 1
        pv_quarter(last, 0, 0, vslabs[last], all_pts[last])
        pv_quarter(last, 0, 1, vslabs[last], all_pts[last])
        finish_head(last, 0, yts[last])
        # WoT streams into recycled PT slots ([128, KT_PER*N] tiles)
        cpt = (KT_PER * N) // EMB      # Wo chunks per PT-sized tile
        n_wt = (NPAIR + cpt - 1) // cpt
        wot_tiles = []
        for t in range(n_wt):
            wt = ptp.tile([128, KT_PER * N], dt_o, name=f"wotT{t}", tag="pt")
            nct = min(cpt, NPAIR - t * cpt)
            nc.sync.dma_start(
                out=wt[:, 0:nct * EMB].rearrange("p (c e) -> p c e", e=EMB),
                in_=WoT_d[t * cpt * 128:(t * cpt + nct) * 128, :]
                .rearrange("(c p) e -> p c e", p=128))
            wot_tiles.append(wt)
        pv_quarter(last, 1, 0, vslabs[last], all_pts[last])
        pv_quarter(last, 1, 1, vslabs[last], all_pts[last])
        finish_head(last, 1, yts[last])
        bcast_mul(last, yts[last])

        def wot_ap(p):
            return wot_tiles[p // cpt][:, (p % cpt) * EMB:
                                       (p % cpt + 1) * EMB]

        for i in range(NT):
            ops = mps.tile([128, EMB], F32, name=f"ops{i}", tag="m")
            for p in range(NPAIR):
                wchunk = wot_ap(p)
                for (s, w) in ech:
                    nc.tensor.matmul(
                        ops[:, s:s + w],
                        yts[p][:, i * 128:(i + 1) * 128],
                        wchunk[:, s:s + w],
                        start=(p == 0), stop=False,
                    )
            for (s, w) in ech:
                nc.tensor.matmul(ops[:, s:s + w], ones_t[:], bo_t[:, s:s + w],
                                 start=False, stop=True)
            osb = xp.tile([128, EMB], F32, name=f"osb{i}", tag="x")
            nc.vector.tensor_copy(osb[:], ops[:])
            nc.sync.dma_start(out=Y_d[i * 128:(i + 1) * 128, :], in_=osb[:])


def build_program(N=1024, EMB=1024, NH=16, n_cores=8, fast=DEFAULT_FAST,
                  repeat=1, trace_sim=False):
    dt_w = F32R if "proj" in fast else F32
    dt_o = F32R if "outp" in fast else F32
    dt_b = F32R if "bias" in fast else F32
    dt_r = F32R if "bcast" in fast else F32
    nc = bacc.Bacc("TRN2", target_bir_lowering=False, debug=False,
                   num_devices=n_cores)
    aps = (
        nc.dram_tensor("X", [N, EMB], F32, kind="ExternalInput").ap(),
        nc.dram_tensor("WqT2", [128, 128], dt_w, kind="ExternalInput").ap(),
        nc.dram_tensor("WkT2", [128, 128], dt_w, kind="ExternalInput").ap(),
        nc.dram_tensor("WvT2", [128, 128], dt_w, kind="ExternalInput").ap(),
        nc.dram_tensor("WoT", [EMB, EMB], dt_o, kind="ExternalInput").ap(),
        nc.dram_tensor("bo", [1, EMB], dt_b, kind="ExternalInput").ap(),
        nc.dram_tensor("sel", [NH, EMB], dt_r, kind="ExternalInput").ap(),
        nc.dram_tensor("ones", [1, 128], dt_b, kind="ExternalInput").ap(),
        nc.dram_tensor("iden", [128, 128], F32, kind="ExternalInput").ap(),
        nc.dram_tensor("Y", [N, EMB], F32, kind="ExternalOutput").ap(),
    )
    with tile.TileContext(nc, trace_sim=trace_sim) as tc:
        for rep in range(repeat):
            emit_body(tc, nc, aps, N, EMB, NH, fast, rep)
    nc.compile()
    return nc


def host_consts(Wq, Wk, Wv, Wo, bo, NH=16):
    EMB = NH * 64

    def blk2(W):
        out = np.zeros((128, 128), np.float32)
        out[0:64, 0:64] = W.T
        out[64:128, 64:128] = W.T
        return out

    sel = np.zeros((NH, EMB), np.float32)
    for p in range(NH // 2):
        sel[2 * p, p * 128:p * 128 + 64] = 1.0
        sel[2 * p + 1, p * 128 + 64:p * 128 + 128] = 1.0
    return {
        "WqT2": blk2(np.asarray(Wq, np.float32)),
        "WkT2": blk2(np.asarray(Wk, np.float32)),
        "WvT2": blk2(np.asarray(Wv, np.float32)),
        "WoT": np.ascontiguousarray(np.asarray(Wo, np.float32).T),
        "bo": np.asarray(bo, np.float32).reshape(1, EMB),
        "sel": sel,
        "ones": np.ones((1, 128), np.float32),
        "iden": np.eye(128, dtype=np.float32),
    }


_NC_CACHE = {}


def kernel(X, Wq, Wk, Wv, Wo, bo):
    X = np.asarray(X, np.float32)
    B, N, EMB = X.shape
    NH = EMB // 64
    key = (N, EMB, NH, B)
    if key not in _NC_CACHE:
        _NC_CACHE[key] = build_program(N=N, EMB=EMB, NH=NH, n_cores=B)
    nc = _NC_CACHE[key]
    consts = host_consts(Wq, Wk, Wv, Wo, bo, NH=NH)
    in_maps = [dict(consts, X=np.ascontiguousarray(X[c])) for c in range(B)]
    res = run_bass_kernel_spmd(nc, in_maps, list(range(B)))
    return np.stack([res.results[c]["Y"] for c in range(B)], axis=0)


if __name__ == "__main__":
    rng = np.random.default_rng(0)
    B, N, EMB, NH = 8, 1024, 1024, 16
    X = rng.standard_normal((B, N, EMB), dtype=np.float32)
    Wq = (rng.standard_normal((64, 64), dtype=np.float32) / 8)
    Wk = (rng.standard_normal((64, 64), dtype=np.float32) / 8)
    Wv = (rng.standard_normal((64, 64), dtype=np.float32) / 8)
    Wo = (rng.standard_normal((EMB, EMB), dtype=np.float32) / 32)
    bo = np.zeros(EMB, np.float32)
    Y = kernel(X=X, Wq=Wq, Wk=Wk, Wv=Wv, Wo=Wo, bo=bo)
    print("OK", Y.shape, Y.dtype)



# revision 5
# speedup vs baseline: 382.8928x; 382.8928x over previous
"""Trainium2 Bass kernel for nn_MultiHeadAttention_38027640439053.

Reference computation (per batch b of 8, one NeuronCore each):
    data = X.reshape(n, 16, 64)
    q/k/v = data @ W{q,k,v}.T          (per-head shared 64x64 weights)
    scores = (q @ k.T per head) / 32
    attn = softmax(scores, axis=k)
    Y = (attn @ v).reshape(n, 1024) @ Wo.T + bo

Strategy (batch-parallel over 8 cores, zero collectives), v2:
  - All matmul operands are bf16 (tolerance is 2e-2; bf16 keeps us ~1e-3).
    X is transposed AND cast to bf16 on the host, so no PE transposes.
  - Per pair of heads p (emb columns p*128..p*128+127):
      xt   = XT[p*128:(p+1)*128, :]                     [128 emb, N]
      qt/kt = blockdiag(W.T) @ xt                       [128, N] bf16
      ST[k, q] = K Q^T per head (transposed scores), two heads run
      concurrently in the PE via row-tiling (K=64 each), fp32 PSUM.
      exp() on ScalarE from PSUM with the 1/32 scale folded in, bf16
      out. No max-subtraction (scores sigma ~0.25).
      P@V as YT = V^T P^T with V row-layout + a ones column -> row 64
      of pvps is the softmax denominator D.
  - Normalization fully deferred: D rows collect into rd[16, N]; ONE
    batched reciprocal at the end (the baseline's 16 per-head
    reciprocals each cost 6.5us of DVE FIFO blockage = 105us), then a
    0/1 selector matmul broadcasts 1/D per pair and one multiply per
    yt slab.
  - Software pipelining: pair p's score loop is ACT-bound (exp is
    ~2.3x the PE's score time), so pair p-1's P@V and pair p+1's
    projections fill the PE.
  - Output projection accumulates 8 emb chunks per n-tile in PSUM;
    bias via rank-1 (ones x bo) matmul in the same accumulation group.
"""

import numpy as np
import ml_dtypes

import concourse.bacc as bacc
import concourse.mybir as mybir
import concourse.tile as tile
from concourse.bass_utils import run_bass_kernel_spmd

F32 = mybir.dt.float32
BF16 = mybir.dt.bfloat16

EXP = mybir.ActivationFunctionType.Exp


def emit_body(tc, nc, aps, N, EMB, NH, rep):
    NPAIR = NH // 2
    KT = N // 128        # k tiles
    NT = N // 128        # n tiles
    assert EMB == NPAIR * 128
    scale = 1.0 / float(np.sqrt(EMB))
    qch = [(s, min(512, N - s)) for s in range(0, N, 512)]

    XT_d, Wq2_d, Wk2_d, Wv2_d, WoT_d, bo_d, sel_d, ones_d, Y_d = aps

    with (
        tc.tile_pool(name=f"consts{rep}", bufs=1) as consts,
        tc.tile_pool(name=f"xtp{rep}", bufs=2) as xtp,
        tc.tile_pool(name=f"qkp{rep}", bufs=4) as qkp,
        tc.tile_pool(name=f"vp{rep}", bufs=2) as vp,
        tc.tile_pool(name=f"ptp{rep}", bufs=2) as ptp,
        tc.tile_pool(name=f"ytp{rep}", bufs=NPAIR) as ytp,
        tc.tile_pool(name=f"rdp{rep}", bufs=1) as rdp,
        tc.tile_pool(name=f"dhp{rep}", bufs=2) as dhp,
        tc.tile_pool(name=f"osp{rep}", bufs=2) as osp,
        tc.tile_pool(name=f"stps{rep}", bufs=2, space="PSUM") as stps,
        tc.tile_pool(name=f"pvps{rep}", bufs=1, space="PSUM") as pvp,
        tc.tile_pool(name=f"mps{rep}", bufs=2, space="PSUM") as mps,
    ):
        # ---- constants ----
        wq2 = consts.tile([128, 128], BF16, name="wq2", tag="wq2")
        nc.sync.dma_start(out=wq2[:], in_=Wq2_d[:])
        wk2 = consts.tile([128, 128], BF16, name="wk2", tag="wk2")
        nc.sync.dma_start(out=wk2[:], in_=Wk2_d[:])
        wv2 = consts.tile([128, 128], BF16, name="wv2", tag="wv2")
        nc.sync.dma_start(out=wv2[:], in_=Wv2_d[:])
        bo_t = consts.tile([1, EMB], BF16, name="bo_t", tag="bo_t")
        nc.sync.dma_start(out=bo_t[:], in_=bo_d[:])
        ones_t = consts.tile([1, 128], BF16, name="ones_t", tag="ones_t")
        nc.sync.dma_start(out=ones_t[:], in_=ones_d[:])
        sel_t = consts.tile([NH, EMB], BF16, name="sel_t", tag="sel_t")
        nc.sync.dma_start(out=sel_t[:], in_=sel_d[:])

        rd = rdp.tile([NH, N], BF16, name="rd", tag="rd")
        rdinv = rdp.tile([NH, N], BF16, name="rdinv", tag="rdinv")

        xts = {}

        def load_xt(p):
            xt = xtp.tile([128, N], BF16, name=f"xt{p}", tag="xt")
            nc.sync.dma_start(out=xt[:], in_=XT_d[p * 128:(p + 1) * 128, :])
            xts[p] = xt

        def proj_qk(p):
            xt = xts[p]
            qt = qkp.tile([128, N], BF16, name=f"qt{p}", tag="qk")
            kt = qkp.tile([128, N], BF16, name=f"kt{p}", tag="qk")
            for dst, w in ((qt, wq2), (kt, wk2)):
                for (s, ww) in qch:
                    ps = mps.tile([128, 512], F32, name=f"prj{p}_{s}", tag="m")
                    nc.tensor.matmul(ps[:], w[:], xt[:, s:s + ww])
                    nc.vector.tensor_copy(dst[:, s:s + ww], ps[:])
            return qt, kt

        def proj_v(p):
            # V in row layout: [n, 2 heads x (64 dims + ones col)]
            xt = xts.pop(p)
            vslab = vp.tile([128, KT * 130], BF16, name=f"vslab{p}", tag="v")
            for half in (0, 1):
                ps = mps.tile([128, 512], F32, name=f"vps{p}_{half}", tag="m")
                for i in range(4):
                    j = half * 4 + i
                    nc.tensor.matmul(ps[:, i * 128:(i + 1) * 128],
                                     xt[:, j * 128:(j + 1) * 128], wv2[:])
                vv = vslab[:, half * 4 * 130:(half * 4 + 4) * 130] \
                    .rearrange("p (j c) -> p j c", c=130)
                vs = ps[:].rearrange("p (j c) -> p j c", c=128)
                nc.vector.tensor_copy(vv[:, :, 0:64], vs[:, :, 0:64])
                nc.vector.tensor_copy(vv[:, :, 65:129], vs[:, :, 64:128])
            v4 = vslab[:].rearrange("p (j k c) -> p j k c", k=2, c=65)
            nc.vector.memset(v4[:, :, :, 64:65], 1.0)
            return vslab

        def st_exp(p, ktile, qt, kt, pt):
            """Transposed scores + exp for one k-tile, both heads."""
            for head in (0, 1):
                r0 = head * 64
                st = stps.tile([128, N], F32, name=f"st{p}_{ktile}_{head}",
                               tag="st")
                for (s, w) in qch:
                    nc.tensor.matmul(
                        st[:, s:s + w],
                        kt[r0:r0 + 64, ktile * 128:(ktile + 1) * 128],
                        qt[r0:r0 + 64, s:s + w],
                    )
                dst = pt[:, (ktile * 2 + head) * N:(ktile * 2 + head + 1) * N]
                nc.scalar.activation(dst, st[:], EXP, scale=scale)

        pv_state = {}

        def pv_half(p, head, half, vslab, pt):
            """8 accumulating matmuls: k-tiles [half*KT/2, (half+1)*KT/2)."""
            if half == 0:
                pv_state[(p, head)] = pvp.tile(
                    [65, N], F32, name=f"pvps{p}_{head}", tag="pv")
            pvps = pv_state[(p, head)]
            k0, k1 = half * (KT // 2), (half + 1) * (KT // 2)
            for ktile in range(k0, k1):
                lhs = vslab[:, ktile * 130 + head * 65:
                            ktile * 130 + head * 65 + 65]
                base = (ktile * 2 + head) * N
                for (s, w) in qch:
                    nc.tensor.matmul(
                        pvps[:, s:s + w], lhs,
                        pt[:, base + s:base + s + w],
                        start=(ktile == 0), stop=(ktile == KT - 1),
                    )

        def finish_head(p, head, yt):
            pvps = pv_state.pop((p, head))
            nc.vector.tensor_copy(yt[head * 64:head * 64 + 64, :],
                                  pvps[0:64, :])
            # softmax denominator row -> rd (reciprocal deferred+batched).
            # DVE writes must be 32-partition aligned, so stage at
            # partition 0 and DMA into the rd row.
            dh = dhp.tile([1, N], BF16, name=f"dh{p}_{head}", tag="dh")
            nc.vector.tensor_copy(dh[:], pvps[64:65, :])
            nc.sync.dma_start(out=rd[2 * p + head:2 * p + head + 1, :],
                              in_=dh[:])

        # ---------------- pipelined pair loop ----------------
        yts = []
        pts = {}
        vslabs = {}

        load_xt(0)
        cur_qt, cur_kt = proj_qk(0)
        vslabs[0] = proj_v(0)
        nxt = {}
        for p in range(NPAIR):
            pt = ptp.tile([128, 2 * KT * N], BF16, name=f"pt{p}", tag="pt")
            pts[p] = pt
            yts.append(ytp.tile([128, N], BF16, name=f"yt{p}", tag="yt"))

            sched = {k: [] for k in range(KT)}
            if p > 0:
                po, vo, po_pt, yo = p - 1, vslabs[p - 1], pts[p - 1], yts[p - 1]
                tasks = [
                    lambda: pv_half(po, 0, 0, vo, po_pt),
                    lambda: (pv_half(po, 0, 1, vo, po_pt),
                             finish_head(po, 0, yo)),
                    lambda: pv_half(po, 1, 0, vo, po_pt),
                    lambda: (pv_half(po, 1, 1, vo, po_pt),
                             finish_head(po, 1, yo)),
                ]
                for j, pos in enumerate((0, 2, 4, 6)):
                    sched[pos].append(tasks[j])
            if p + 1 < NPAIR:
                pn = p + 1
                tasks = [
                    lambda: load_xt(pn),
                    lambda: nxt.__setitem__("qk", proj_qk(pn)),
                    lambda: vslabs.__setitem__(pn, proj_v(pn)),
                ]
                for j, pos in enumerate((1, 3, 5)):
                    sched[pos].append(tasks[j])
            for ktile in range(KT):
                for t in sched[ktile]:
                    t()
                st_exp(p, ktile, cur_qt, cur_kt, pt)
            if p - 1 >= 0:
                del vslabs[p - 1], pts[p - 1]
            if p + 1 < NPAIR:
                cur_qt, cur_kt = nxt["qk"]

        # ---------------- tail ----------------
        last = NPAIR - 1
        pv_half(last, 0, 0, vslabs[last], pts[last])
        # Wo^T streams into a recycled PT slot while last pair's PV runs
        wot = ptp.tile([128, NPAIR * EMB], BF16, name="wot", tag="pt")
        nc.sync.dma_start(
            out=wot[:].rearrange("p (c e) -> p c e", e=EMB),
            in_=WoT_d[:].rearrange("(c p) e -> p c e", p=128))
        pv_half(last, 0, 1, vslabs[last], pts[last])
        finish_head(last, 0, yts[last])
        pv_half(last, 1, 0, vslabs[last], pts[last])
        pv_half(last, 1, 1, vslabs[last], pts[last])
        finish_head(last, 1, yts[last])

        # batched softmax normalization: rdinv = 1/rd, broadcast per pair
        with nc.allow_low_precision(reason="bf16 softmax denom"):
            nc.vector.reciprocal(rdinv[:], rd[:])
        for p in range(NPAIR):
            for (s, w) in qch:
                bps = mps.tile([128, 512], F32, name=f"bps{p}_{s}", tag="m")
                nc.tensor.matmul(bps[:],
                                 sel_t[:, p * 128:(p + 1) * 128],
                                 rdinv[:, s:s + w])
                nc.vector.tensor_mul(yts[p][:, s:s + w],
                                     yts[p][:, s:s + w], bps[:])

        # output projection: Y[i-tile] = sum_p yt_p^T @ WoT_p + bo
        for i in range(NT):
            osb = osp.tile([128, EMB], F32, name=f"osb{i}", tag="o")
            for (s, w) in qch:
                ops = mps.tile([128, 512], F32, name=f"ops{i}_{s}", tag="m")
                for p in range(NPAIR):
                    nc.tensor.matmul(
                        ops[:],
                        yts[p][:, i * 128:(i + 1) * 128],
                        wot[:, p * EMB + s:p * EMB + s + w],
                        start=(p == 0), stop=False,
                    )
                nc.tensor.matmul(ops[:], ones_t[:], bo_t[:, s:s + w],
                                 start=False, stop=True)
                nc.vector.tensor_copy(osb[:, s:s + w], ops[:])
            nc.sync.dma_start(out=Y_d[i * 128:(i + 1) * 128, :], in_=osb[:])


def build_program(N=1024, EMB=1024, NH=16, n_cores=8, repeat=1,
                  trace_sim=False):
    nc = bacc.Bacc("TRN2", target_bir_lowering=False, debug=False,
                   num_devices=n_cores)
    aps = (
        nc.dram_tensor("XT", [EMB, N], BF16, kind="ExternalInput").ap(),
        nc.dram_tensor("Wq2", [128, 128], BF16, kind="ExternalInput").ap(),
        nc.dram_tensor("Wk2", [128, 128], BF16, kind="ExternalInput").ap(),
        nc.dram_tensor("Wv2", [128, 128], BF16, kind="ExternalInput").ap(),
        nc.dram_tensor("WoT", [EMB, EMB], BF16, kind="ExternalInput").ap(),
        nc.dram_tensor("bo", [1, EMB], BF16, kind="ExternalInput").ap(),
        nc.dram_tensor("sel", [NH, EMB], BF16, kind="ExternalInput").ap(),
        nc.dram_tensor("ones", [1, 128], BF16, kind="ExternalInput").ap(),
        nc.dram_tensor("Y", [N, EMB], F32, kind="ExternalOutput").ap(),
    )
    with tile.TileContext(nc, trace_sim=trace_sim) as tc:
        for rep in range(repeat):
            emit_body(tc, nc, aps, N, EMB, NH, rep)
    nc.compile()
    return nc


def host_consts(Wq, Wk, Wv, Wo, bo, NH=16):
    EMB = NH * 64
    bf = ml_dtypes.bfloat16

    def blk2(W):
        out = np.zeros((128, 128), np.float32)
        out[0:64, 0:64] = W.T
        out[64:128, 64:128] = W.T
        return out.astype(bf)

    sel = np.zeros((NH, EMB), np.float32)
    for p in range(NH // 2):
        sel[2 * p, p * 128:p * 128 + 64] = 1.0
        sel[2 * p + 1, p * 128 + 64:p * 128 + 128] = 1.0
    return {
        "Wq2": blk2(np.asarray(Wq, np.float32)),
        "Wk2": blk2(np.asarray(Wk, np.float32)),
        "Wv2": blk2(np.asarray(Wv, np.float32)),
        "WoT": np.ascontiguousarray(np.asarray(Wo, np.float32).T).astype(bf),
        "bo": np.asarray(bo, np.float32).reshape(1, EMB).astype(bf),
        "sel": sel.astype(bf),
        "ones": np.ones((1, 128), np.float32).astype(bf),
    }


_NC_CACHE = {}


def kernel(X, Wq, Wk, Wv, Wo, bo):
    X = np.asarray(X, np.float32)
    B, N, EMB = X.shape
    NH = EMB // 64
    key = (N, EMB, NH, B)
    if key not in _NC_CACHE:
        _NC_CACHE[key] = build_program(N=N, EMB=EMB, NH=NH, n_cores=B)
    nc = _NC_CACHE[key]
    consts = host_consts(Wq, Wk, Wv, Wo, bo, NH=NH)
    bf = ml_dtypes.bfloat16
    in_maps = [
        dict(consts, XT=np.ascontiguousarray(X[c].T).astype(bf))
        for c in range(B)
    ]
    res = run_bass_kernel_spmd(nc, in_maps, list(range(B)))
    return np.stack([res.results[c]["Y"] for c in range(B)], axis=0)


if __name__ == "__main__":
    rng = np.random.default_rng(0)
    B, N, EMB, NH = 8, 1024, 1024, 16
    X = rng.standard_normal((B, N, EMB), dtype=np.float32)
    Wq = (rng.standard_normal((64, 64), dtype=np.float32) / 8)
    Wk = (rng.standard_normal((64, 64), dtype=np.float32) / 8)
    Wv = (rng.standard_normal((64, 64), dtype=np.float32) / 8)
    Wo = (rng.standard_normal((EMB, EMB), dtype=np.float32) / 32)
    bo = np.zeros(EMB, np.float32)
    Y = kernel(X=X, Wq=Wq, Wk=Wk, Wv=Wv, Wo=Wo, bo=bo)

    # numpy reference check
    def ref(X, Wq, Wk, Wv, Wo, bo):
        b, n, d = X.shape
        hd = Wq.shape[0]
        h = d // hd
        data = X.reshape(b, n, h, hd)
        q = np.einsum('bnhd,ed->bnhe', data, Wq)
        k = np.einsum('bnhd,ed->bnhe', data, Wk)
        v = np.einsum('bnhd,ed->bnhe', data, Wv)
        s = np.einsum('bqhd,bkhd->bhqk', q, k) / np.sqrt(d)
        s = np.exp(s - s.max(-1, keepdims=True))
        attn = s / s.sum(-1, keepdims=True)
        Yr = np.einsum('bhqk,bkhd->bqhd', attn, v).reshape(b, n, d)
        return Yr @ Wo.T + bo

    E = ref(X, Wq, Wk, Wv, Wo, bo)
    err = np.abs(Y - E).max() / np.abs(E).max()
    print("OK", Y.shape, Y.dtype, "rel err", err)
